# revision 11
# baseline (speedup 1.0000x reference)
"""ChunkKVCompressor Trainium2 kernel.

Data-parallel over batch: core i handles batch element i (B=8 across 8 cores).
Per core:
  1. Stream keys/values in 8 groups of 512 tokens; c = k + v (DVE), rounded
     to the matmul dtype (float32r by default).
  2. PE-transpose c into [D, tokens] layout.
  3. hT = W1.T @ cT via float32r matmuls (1 cyc/row at N=512), fp32 PSUM.
  4. relu(0.5*hT + b1) fused on the scalar engine (ACT).
  5. Per-chunk token sums (DVE segment reduce), then scores += W2.T @ sums
     into one persistent PSUM row [1, 64]  (mean-scale/b2 dropped: order-
     preserving affine transform doesn't change top-k).
  6. On-device top-32-of-64 by rank: rank[i] = #{j: s[j]>s[i]} + ties(j<i);
     keep = rank<32; slots = exclusive-cumsum; slot->chunk map via tiny
     fp32 matmuls; gather row indices built on DVE.
  7. Gather: indirect DMA (DRAM->SBUF) of selected 256KB chunks viewed as
     128 rows x 512 f32, then contiguous HWDGE writes to the outputs.
"""

import sys

if "/opt/trn_rl_repo" not in sys.path:
    sys.path.insert(0, "/opt/trn_rl_repo")

import numpy as np
from contextlib import ExitStack

B, T, D, H = 8, 4096, 1024, 512
L = 64  # chunk length (tokens)
NCH = T // L  # 64 chunks
KEEP = 32  # chunks kept per batch
NG = 8  # token groups
GT = T // NG  # 512 tokens per group
NJ = D // 128  # 8 d-blocks
NHB = H // 128  # 4 h-blocks
NS = GT // 128  # 4 token sub-tiles per group
CPG = NCH // NG  # 8 chunks per group
GCH = 8  # chunks per gather stage
MM_MODE = "f32r"  # "f32r" | "f32" | "bf16"

_CACHE = {}


def _build(mode=MM_MODE):
    import concourse.bass as bass
    import concourse.tile as tile
    from concourse import bacc, mybir

    f32 = mybir.dt.float32
    i32 = mybir.dt.int32
    mm_dt = {
        "f32r": mybir.dt.float32r,
        "f32": f32,
        "bf16": mybir.dt.bfloat16,
    }[mode]

    nc = bacc.Bacc("TRN2", target_bir_lowering=False, debug=False, num_devices=B)

    keys = nc.dram_tensor("keys", [T, D], f32, kind="ExternalInput").ap()
    values = nc.dram_tensor("values", [T, D], f32, kind="ExternalInput").ap()
    w1 = nc.dram_tensor("w1", [D, H], mm_dt, kind="ExternalInput").ap()
    b1c = nc.dram_tensor("b1c", [128, NHB], f32, kind="ExternalInput").ap()
    w2c = nc.dram_tensor("w2c", [128, NHB], f32, kind="ExternalInput").ap()
    ident_d = nc.dram_tensor("ident", [128, 128], mm_dt, kind="ExternalInput").ap()
    ones_d = nc.dram_tensor("ones_row", [1, 128], f32, kind="ExternalInput").ap()
    jlt_d = nc.dram_tensor("jlt", [NCH, NCH], f32, kind="ExternalInput").ap()
    tbm_d = nc.dram_tensor("tbm", [NCH, NCH], f32, kind="ExternalInput").ap()
    iota32_d = nc.dram_tensor("iota32", [NCH, KEEP], f32, kind="ExternalInput").ap()
    iota64_d = nc.dram_tensor("iota64", [NCH, 1], f32, kind="ExternalInput").ap()
    iotap16_d = nc.dram_tensor("iotap16", [128, 1], f32, kind="ExternalInput").ap()
    ck = nc.dram_tensor("ck", [KEEP * L, D], f32, kind="ExternalOutput").ap()
    cv = nc.dram_tensor("cv", [KEEP * L, D], f32, kind="ExternalOutput").ap()

    with tile.TileContext(nc) as tc, ExitStack() as ctx:
        wp = ctx.enter_context(tc.tile_pool(name="wp", bufs=1))
        kp = ctx.enter_context(tc.tile_pool(name="kp", bufs=2))
        vp = ctx.enter_context(tc.tile_pool(name="vp", bufs=2))
        # ctile (scoring stage) and gather staging share slots via one pool
        cp = ctx.enter_context(tc.tile_pool(name="cp", bufs=2))
        ctp = ctx.enter_context(tc.tile_pool(name="ctp", bufs=2))
        htp = ctx.enter_context(tc.tile_pool(name="htp", bufs=4))
        csp = ctx.enter_context(tc.tile_pool(name="csp", bufs=8))
        selp = ctx.enter_context(tc.tile_pool(name="selp", bufs=2))
        pst_p = ctx.enter_context(tc.tile_pool(name="pst", bufs=2, space="PSUM"))
        ph_p = ctx.enter_context(tc.tile_pool(name="ph", bufs=2, space="PSUM"))
        sc_p = ctx.enter_context(tc.tile_pool(name="sc", bufs=1, space="PSUM"))
        psel = ctx.enter_context(tc.tile_pool(name="psel", bufs=2, space="PSUM"))

        # --- constants / weights to SBUF -------------------------------
        w1sb = wp.tile([128, NJ, H], mm_dt)  # [p, j, h]; d = j*128 + p
        nc.sync.dma_start(w1sb[:], w1.rearrange("(j p) h -> p j h", p=128))
        b1sb = wp.tile([128, NHB], f32)
        nc.sync.dma_start(b1sb[:], b1c[:])
        w2sb = wp.tile([128, NHB], f32)
        nc.sync.dma_start(w2sb[:], w2c[:])
        ident_sb = wp.tile([128, 128], mm_dt)
        nc.sync.dma_start(ident_sb[:], ident_d[:])
        ones_sb = wp.tile([1, 128], f32)
        nc.sync.dma_start(ones_sb[:], ones_d[:])
        jlt_sb = wp.tile([NCH, NCH], f32)
        nc.sync.dma_start(jlt_sb[:], jlt_d[:])
        tbm_sb = wp.tile([NCH, NCH], f32)
        nc.sync.dma_start(tbm_sb[:], tbm_d[:])
        iota32_sb = wp.tile([NCH, KEEP], f32)
        nc.sync.dma_start(iota32_sb[:], iota32_d[:])
        iota64_sb = wp.tile([NCH, 1], f32)
        nc.sync.dma_start(iota64_sb[:], iota64_d[:])
        iotap16_sb = wp.tile([128, 1], f32)
        nc.sync.dma_start(iotap16_sb[:], iotap16_d[:])

        keys_g = keys.rearrange("(g s p) d -> g p s d", s=NS, p=128)
        values_g = values.rearrange("(g s p) d -> g p s d", s=NS, p=128)

        scores_ps = sc_p.tile([1, NCH], f32, space="PSUM")

        # --- scoring ----------------------------------------------------
        for g in range(NG):
            ktile = kp.tile([128, NS, D], f32, tag="ktile")
            nc.sync.dma_start(ktile[:], keys_g[g])
            vtile = vp.tile([128, NS, D], f32, tag="vtile")
            nc.sync.dma_start(vtile[:], values_g[g])
            ctile = cp.tile([128, NS, D], mm_dt, tag="cstage")
            nc.vector.tensor_add(ctile[:], ktile[:], vtile[:])

            cT = ctp.tile([128, NJ, GT], mm_dt, tag="cT")  # [p=d%128, j, t]
            for j in range(NJ):
                pst = pst_p.tile([128, GT], mm_dt, tag="pst")
                for s in range(NS):
                    nc.tensor.transpose(
                        pst[:, 128 * s : 128 * (s + 1)],
                        ctile[:, s, 128 * j : 128 * (j + 1)],
                        ident_sb[:],
                    )
                nc.vector.tensor_copy(cT[:, j, :], pst[:])

            for hb in range(NHB):
                ph = ph_p.tile([128, GT], f32, tag="ph")
                for j in range(NJ):
                    nc.tensor.matmul(
                        ph[:],
                        w1sb[:, j, 128 * hb : 128 * (hb + 1)],
                        cT[:, j, :],
                        start=(j == 0),
                        stop=(j == NJ - 1),
                    )
                ht = htp.tile([128, GT], f32, tag="ht")
                nc.scalar.activation(
                    ht[:],
                    ph[:],
                    mybir.ActivationFunctionType.Relu,
                    bias=b1sb[:, hb : hb + 1],
                    scale=0.5,
                )
                csum = csp.tile([128, CPG], f32, tag="csum")
                nc.vector.tensor_reduce(
                    csum[:],
                    ht.rearrange("p (c l) -> p c l", l=L),
                    axis=mybir.AxisListType.X,
                    op=mybir.AluOpType.add,
                )
                nc.tensor.matmul(
                    scores_ps[0:1, CPG * g : CPG * (g + 1)],
                    w2sb[:, hb : hb + 1],
                    csum[:],
                    start=(hb == 0),
                    stop=(hb == NHB - 1),
                )

        # --- top-32 selection (all fp32, exact) ------------------------
        scores_sb = selp.tile([1, NCH], f32, tag="sel_s")
        nc.vector.tensor_copy(scores_sb[:], scores_ps[:])

        sT_ps = psel.tile([NCH, 1], f32, space="PSUM", tag="psel")
        nc.tensor.matmul(sT_ps[:], scores_sb[:], ones_sb[:, 0:1])
        sT_sb = selp.tile([NCH, 1], f32, tag="sel_sT")
        nc.vector.tensor_copy(sT_sb[:], sT_ps[:])

        r_ps = psel.tile([NCH, NCH], f32, space="PSUM", tag="psel")
        nc.tensor.matmul(r_ps[:], ones_sb[:, :NCH], scores_sb[:])
        r_sb = selp.tile([NCH, NCH], f32, tag="sel_r")
        nc.vector.tensor_copy(r_sb[:], r_ps[:])

        g_sb = selp.tile([NCH, NCH], f32, tag="sel_g")
        nc.vector.tensor_scalar(
            g_sb[:], r_sb[:], sT_sb[:], None, op0=mybir.AluOpType.is_gt
        )
        eq_sb = selp.tile([NCH, NCH], f32, tag="sel_eq")
        nc.vector.tensor_scalar(
            eq_sb[:], r_sb[:], sT_sb[:], None, op0=mybir.AluOpType.is_equal
        )
        tie_sb = selp.tile([NCH, NCH], f32, tag="sel_tie")
        nc.vector.tensor_mul(tie_sb[:], eq_sb[:], tbm_sb[:])
        nc.vector.tensor_add(g_sb[:], g_sb[:], tie_sb[:])
        rank_sb = selp.tile([NCH, 1], f32, tag="sel_rank")
        nc.vector.tensor_reduce(
            rank_sb[:], g_sb[:], axis=mybir.AxisListType.X, op=mybir.AluOpType.add
        )
        keep_sb = selp.tile([NCH, 1], f32, tag="sel_keep")
        nc.vector.tensor_scalar(
            keep_sb[:], rank_sb[:], float(KEEP) - 0.5, None, op0=mybir.AluOpType.is_lt
        )

        dest_ps = psel.tile([NCH, 1], f32, space="PSUM", tag="psel")
        nc.tensor.matmul(dest_ps[:], jlt_sb[:], keep_sb[:])
        dest_sb = selp.tile([NCH, 1], f32, tag="sel_dest")
        nc.vector.tensor_copy(dest_sb[:], dest_ps[:])

        sel1_sb = selp.tile([NCH, KEEP], f32, tag="sel_m")
        nc.vector.tensor_scalar(
            sel1_sb[:], iota32_sb[:], dest_sb[:], None, op0=mybir.AluOpType.is_equal
        )
        nc.vector.tensor_scalar(
            sel1_sb[:], sel1_sb[:], keep_sb[:], None, op0=mybir.AluOpType.mult
        )

        # --- gather + write out ----------------------------------------
        # Source viewed as [1024 rows, 4096 elems] (one row = 4 tokens).
        # One indirect DMA moves 8 chunks (2 MB): partition p fetches source
        # row 16*idx[8*d + p//16] + p%16, one index per partition (the only
        # offset-AP shape the HW DGE honors).
        keys_rows = keys.rearrange("(r q) d -> r (q d)", q=4)
        values_rows = values.rearrange("(r q) d -> r (q d)", q=4)
        ck_rows = ck.rearrange("(r q) d -> r (q d)", q=4)
        cv_rows = cv.rearrange("(r q) d -> r (q d)", q=4)

        NDMA = KEEP // GCH  # 4 indirect DMAs per tensor
        rowidx = []
        selrep = selp.tile([NCH, NDMA * 128], f32, tag="sel_rep")
        for d in range(NDMA):
            # selrep[i, 128*d + p] = sel1[i, 8*d + p//16]
            nc.vector.tensor_copy(
                selrep.rearrange("i (d o u) -> i d o u", d=NDMA, u=16)[:, d],
                sel1_sb[:, GCH * d : GCH * (d + 1)].to_broadcast([NCH, GCH, 16]),
            )
        for d in range(NDMA):
            rsel_ps = psel.tile([128, 1], f32, space="PSUM", tag="psel")
            nc.tensor.matmul(
                rsel_ps[:],
                selrep[:, 128 * d : 128 * (d + 1)],
                iota64_sb[:],
            )
            rowf = selp.tile([128, 1], f32, tag="sel_rowf")
            nc.vector.tensor_scalar(
                rowf[:],
                rsel_ps[:],
                16.0,
                iotap16_sb[:],
                op0=mybir.AluOpType.mult,
                op1=mybir.AluOpType.add,
            )
            ridx = selp.tile([128, 1], i32, tag=f"sel_rowi{d}")
            nc.vector.tensor_copy(ridx[:], rowf[:])
            rowidx.append(ridx)

        for src_rows, dst_rows in ((keys_rows, ck_rows), (values_rows, cv_rows)):
            for d in range(NDMA):
                gt = cp.tile([128, 4096], f32, tag="cstage")
                nc.gpsimd.indirect_dma_start(
                    out=gt[:],
                    out_offset=None,
                    in_=src_rows,
                    in_offset=bass.IndirectOffsetOnAxis(ap=rowidx[d][:], axis=0),
                )
                nc.sync.dma_start(dst_rows[128 * d : 128 * (d + 1), :], gt[:])

    nc.compile()
    return nc


def _host_consts(W1, b1, W2, mode=MM_MODE):
    f32 = np.float32
    if mode == "bf16":
        import ml_dtypes

        cdt = ml_dtypes.bfloat16
    else:
        cdt = f32
    consts = {
        "w1": np.ascontiguousarray(W1, dtype=cdt),
        "b1c": np.ascontiguousarray(np.asarray(b1, f32).reshape(NHB, 128).T),
        "w2c": np.ascontiguousarray(np.asarray(W2, f32)[:, 0].reshape(NHB, 128).T),
        "ident": np.eye(128, dtype=cdt),
        "ones_row": np.ones((1, 128), dtype=f32),
        "jlt": (np.arange(NCH)[:, None] < np.arange(NCH)[None, :]).astype(f32),
        "tbm": (np.arange(NCH)[None, :] < np.arange(NCH)[:, None]).astype(f32),
        "iota32": np.ascontiguousarray(
            np.broadcast_to(np.arange(KEEP, dtype=f32), (NCH, KEEP))
        ),
        "iota64": np.arange(NCH, dtype=f32).reshape(NCH, 1),
        "iotap16": (np.arange(128, dtype=f32) % 16).reshape(128, 1),
    }
    return consts


def get_nc(mode=MM_MODE):
    key = ("nc", mode)
    if key not in _CACHE:
        _CACHE[key] = _build(mode)
    return _CACHE[key]


def kernel(keys, values, W1, b1, W2, b2):
    from concourse.bass_utils import run_bass_kernel_spmd

    nc = get_nc()
    keys = np.asarray(keys)
    values = np.asarray(values)
    consts = _host_consts(np.asarray(W1), np.asarray(b1), np.asarray(W2))
    in_maps = [dict(keys=keys[i], values=values[i], **consts) for i in range(B)]
    res = run_bass_kernel_spmd(nc, in_maps, list(range(B)))
    ck = np.stack([res.results[i]["ck"] for i in range(B)])
    cv = np.stack([res.results[i]["cv"] for i in range(B)])
    return ck, cv
